# revision 1
# baseline (speedup 1.0000x reference)
"""DOSLoss Trainium2 kernel.

Full inputs in, scalar loss out. Internally: pure data-parallel shard of the
batch axis across 8 NeuronCores. Each core streams its shard of cls_score
([8,512,1000]) and n ([8,512,256]) through a Bass/Tile kernel that computes
the two per-(b,k) contractions:

    expsum[b,k] = sum_c exp(cls_score[b,k,c])      (ACT engine, fused accum)
    d2[b,k]     = sum_d (deep_feats[b,d]-n[b,k,d])^2  (DVE sub + fused sq-reduce)

Device layout: k = p*4 + s (partition p, sub-row s) so each partition's DRAM
read is fully contiguous; SBUF result column col = b*4 + s.
The O(B*K) scalar tail (log, sqrt, masked softmax over ragged lengths, target
gather, final sums) runs on host in float64, and the 8 per-core partials are
reduced on host.
"""

import os
import time

import numpy as np

B, KMAX, D, C = 64, 512, 256, 1000
N_CORES = 8
BS = B // N_CORES  # samples per core
P = 128
J = KMAX // P  # k-chunks per sample
NCOL = BS * J  # 32 result columns per core

_CACHE = {}
LAST_RESULTS = None  # BassKernelResults of the most recent device run


def _build_nc():
    import concourse.bacc as bacc
    import concourse.mybir as mybir
    import concourse.tile as tile

    f32 = mybir.dt.float32
    nc = bacc.Bacc("TRN2", target_bir_lowering=False, debug=False)

    cls_t = nc.dram_tensor("cls", [BS, KMAX, C], f32, kind="ExternalInput")
    n_t = nc.dram_tensor("nn", [BS, KMAX, D], f32, kind="ExternalInput")
    fb_t = nc.dram_tensor("fb", [1, BS * D], f32, kind="ExternalInput")
    out_t = nc.dram_tensor("out", [P, 2 * NCOL], f32, kind="ExternalOutput")

    # k = p*J + s  ->  partition p, free dims (s, inner); per-partition rows are
    # fully contiguous in DRAM (16KB for cls, 4KB for n) -> fatter DMA descriptors
    cls_r = cls_t.ap().rearrange("b (p s) c -> b p s c", s=J)
    n_r = n_t.ap().rearrange("b (p s) d -> b p s d", s=J)

    with tile.TileContext(nc) as tc:
        with (
            tc.tile_pool(name="cls_pool", bufs=5) as cls_pool,
            tc.tile_pool(name="n_pool", bufs=5) as n_pool,
            tc.tile_pool(name="scr_pool", bufs=4) as scr_pool,
            tc.tile_pool(name="acc", bufs=1) as acc,
        ):
            import concourse.bass as bass

            fb = acc.tile([P, BS * D], f32)
            # partition-broadcast DMA: step-0 over the partition dim replicates
            # the [1, BS*D] feature row into all 128 partitions. HWDGE accepts
            # this AP, keeping the kernel gpsimd-free (no Pool dge_drain at the
            # kernel tail).
            fb_bcast_src = bass.AP(
                tensor=fb_t.ap().tensor,
                offset=0,
                ap=[[0, P], [1, BS * D]],
            )
            nc.sync.dma_start(out=fb, in_=fb_bcast_src)
            res = acc.tile([P, 2 * NCOL], f32)  # cols [0,32): expsum, [32,64): d2

            last = BS - 1
            for b in range(BS):
                # Samples 0..BS-2: one 2MB cls DMA. Last sample: four 512KB
                # chunk DMAs on an independent tile tag, shrinking the
                # end-of-kernel compute tail to a single exp. Steady-state
                # measures ~52 +/- 4us/iter, at the ~53us DMA-only floor.
                ctile = None
                cchunks = []
                if b != last:
                    ctile = cls_pool.tile([P, J, C], f32, tag="cls")
                    nc.sync.dma_start(out=ctile, in_=cls_r[b])
                ntile = n_pool.tile([P, J, D], f32, tag="nn")
                nc.sync.dma_start(out=ntile, in_=n_r[b])
                if b == last:
                    for j in range(J):
                        cch = cls_pool.tile([P, 1, C], f32, tag="clsch")
                        nc.sync.dma_start(out=cch, in_=cls_r[b][:, j : j + 1, :])
                        cchunks.append(cch)
                for j in range(J):
                    col = b * J + j
                    scr = scr_pool.tile([P, C], f32, tag="scr")
                    nc.scalar.activation(
                        out=scr,
                        in_=ctile[:, j, :] if b != last else cchunks[j][:, 0, :],
                        func=mybir.ActivationFunctionType.Exp,
                        accum_out=res[:, col : col + 1],
                    )
                # NOTE: tensor_tensor_reduce reliably faults the exec unit on
                # this HW/axon stack — use sub + mul + reduce_sum instead.
                # One wide op per sample (all J chunks at once) minimizes DVE
                # instruction count and per-op DRAIN overhead.
                diff4 = scr_pool.tile([P, J, D], f32, tag="diff4")
                nc.vector.tensor_sub(
                    diff4,
                    ntile,
                    fb[:, b * D : (b + 1) * D]
                    .rearrange("p (o d) -> p o d", o=1)
                    .broadcast_to((P, J, D)),
                )
                sq4 = scr_pool.tile([P, J, D], f32, tag="sq4")
                nc.vector.tensor_mul(sq4, diff4, diff4)
                nc.vector.reduce_sum(
                    out=res[:, NCOL + b * J : NCOL + (b + 1) * J],
                    in_=sq4,
                    axis=mybir.AxisListType.X,
                )

            nc.sync.dma_start(out=out_t.ap(), in_=res)

    nc.compile()
    return nc


def _get_nc():
    if "nc" not in _CACHE:
        _CACHE["nc"] = _build_nc()
    return _CACHE["nc"]


def _run_device(in_maps):
    global LAST_RESULTS
    from concourse import bass_utils

    nc = _get_nc()
    trace = bool(int(os.environ.get("DOS_TRACE", "0")))
    last_exc = None
    for attempt in range(3):
        try:
            results = bass_utils.run_bass_kernel_spmd(
                nc, in_maps, core_ids=list(range(N_CORES)), trace=trace
            )
            break
        except Exception as e:
            # transient NRT hiccups (e.g. NRT_EXEC_UNIT_UNRECOVERABLE) can
            # resolve on retry once the runtime recovers the core
            last_exc = e
            time.sleep(5)
    else:
        raise last_exc
    LAST_RESULTS = results
    return [r["out"] for r in results.results]


def kernel(deep_feats, n, w, cls_score, target, lengths):
    deep_feats = np.ascontiguousarray(np.asarray(deep_feats, dtype=np.float32))
    n = np.ascontiguousarray(np.asarray(n, dtype=np.float32))
    w = np.asarray(w, dtype=np.float32)
    cls_score = np.ascontiguousarray(np.asarray(cls_score, dtype=np.float32))
    target = np.asarray(target).astype(np.int64)
    lengths = np.asarray(lengths).astype(np.int64)

    in_maps = []
    for c in range(N_CORES):
        lo, hi = c * BS, (c + 1) * BS
        fb = np.ascontiguousarray(deep_feats[lo:hi].reshape(1, BS * D))
        in_maps.append(
            {
                "cls": np.ascontiguousarray(cls_score[lo:hi]),
                "nn": np.ascontiguousarray(n[lo:hi]),
                "fb": fb,
            }
        )

    outs = _run_device(in_maps)

    # [P, 2*NCOL] per core -> [B, KMAX] expsum / d2, with k = j*128 + p
    expsum = np.empty((B, KMAX), dtype=np.float64)
    d2 = np.empty((B, KMAX), dtype=np.float64)
    for c in range(N_CORES):
        o = outs[c].astype(np.float64)  # [128, 64]
        es = o[:, :NCOL].reshape(P, BS, J).transpose(1, 0, 2).reshape(BS, KMAX)
        dd = o[:, NCOL:].reshape(P, BS, J).transpose(1, 0, 2).reshape(BS, KMAX)
        expsum[c * BS : (c + 1) * BS] = es
        d2[c * BS : (c + 1) * BS] = dd

    # host tail in float64
    lse = np.log(expsum)  # [B, KMAX]
    dist = np.sqrt(np.maximum(d2, 0.0))  # [B, KMAX]
    mask = (np.arange(KMAX)[None, :] < lengths[:, None]).astype(np.float64)
    s = -w.astype(np.float64) * dist
    f_loss = float(np.sum(s * mask))

    smax = np.max(np.where(mask > 0, s, -np.inf), axis=1, keepdims=True)
    e = np.exp(s - smax) * mask
    rho = e / np.sum(e, axis=1, keepdims=True)

    cls_at = cls_score[np.arange(B)[:, None], np.arange(KMAX)[None, :], target[:, None]]
    ce = lse - cls_at.astype(np.float64)
    g_loss = float(np.sum(rho * ce))

    return np.float32(f_loss + g_loss)



# revision 6
# speedup vs baseline: 2.3834x; 2.3834x over previous
"""DOSLoss Trainium2 kernel — ragged-packed, class-on-partition layout.

Full inputs in, scalar loss out. The two heavy per-row contractions run on
device; everything O(B*K) runs on host in float64.

Key ideas vs the naive per-sample kernel:
  * Ragged packing: only the sum(lengths) valid (b,k) rows are uploaded and
    processed (~half of B*Kmax for uniform lengths), load-balanced so every
    core gets ceil(V/8) rows regardless of per-sample lengths.
  * Class-on-partition layout: cls row r is stored as [125 partitions, 8]
    (c = p*8 + j, 1000 = 125*8 exactly). The ACT engine then does pure
    elementwise exp (8 elems/row of engine time, its roofline) with no
    per-instruction accumulator reads. A 2-level DVE pairwise add tree
    (bf16, 2x SIMD mode) folds 8 -> 2, and a ones-weight matmul on the
    otherwise idle PE contracts the 125 partitions into PSUM.
  * d2 via h = (n - 2f)*n summed by PE: host uploads n rows ([128, R, 2],
    d = p*2 + j) and 2*deep_feats replicated per row; DVE computes h in two
    tensor ops; PE ones-matmuls accumulate sum(n^2) - 2*f.n into PSUM in the
    same accumulation group. Host adds ||f||^2 (exact, fp64).
  * dtypes: cls in fp8 e4m3 (exp(x) feeds a 1000-term sum; rounding is
    ~0.04 absolute on the logit -> ~2e-3 on lse, irrelevant at the 2e-2
    gate), n/2f in bf16. Halves/quarters DMA bytes; ACT cost is dtype-blind.
  * Per-block PE results land in per-partition rows of two persistent PSUM
    tiles ([NBLK, R_blk], block b -> partition b); two partition-parallel
    DVE copies at the tail evacuate them to SBUF for one small output DMA
    (DMA cannot read PSUM directly).

Per-block (424 rows) engine budget from the TimelineSim cost model:
ACT 2.92us (bottleneck), DMA 2.2us, DVE ~2.2us, PE ~1.4us.
"""

import os
import time

import numpy as np

B, KMAX, D, C = 64, 512, 256, 1000
N_CORES = 8
PC = 125  # class partitions: C = PC * 8
JC = 8
PD = 128  # d partitions: D = PD * 2
JD = 2
MAX_RBLK = 448  # matmul moving-dim cap is 512; keep a multiple of 8 below it

_CACHE = {}
LAST_RESULTS = None  # BassKernelResults of the most recent device run


def _plan(v_max):
    """Block plan for v_max rows per core: tuple of block row counts.

    Small first block shortens pipeline fill (first exp starts after a small
    cls DMA); small last block shortens the post-ACT tail chain (fold +
    matmul + PSUM copies of the final block). Middle blocks bounded by the
    512-row matmul moving-dim cap.
    """
    if v_max <= 512:
        return (-(-v_max // 8) * 8,)
    head, tail = 128, 96
    rem = v_max - head - tail
    nmid = max(1, -(-rem // MAX_RBLK))
    mid = -(-rem // (nmid * 8)) * 8
    return (head,) + (mid,) * nmid + (tail,)


def _build_nc(blocks):
    import concourse.bacc as bacc
    import concourse.mybir as mybir
    import concourse.tile as tile

    f32 = mybir.dt.float32
    bf16 = mybir.dt.bfloat16
    f8 = mybir.dt.float8e4
    r_pad = sum(blocks)

    nc = bacc.Bacc("TRN2", target_bir_lowering=False, debug=False)

    cls_t = nc.dram_tensor("cls8", [PC, r_pad, JC], f8, kind="ExternalInput")
    n_t = nc.dram_tensor("nn2t", [PD, r_pad, JD], bf16, kind="ExternalInput")
    f_t = nc.dram_tensor("f2r", [PD, r_pad, JD], bf16, kind="ExternalInput")
    out_t = nc.dram_tensor("out", [2, r_pad], f32, kind="ExternalOutput")

    with tile.TileContext(nc) as tc:
        with (
            tc.tile_pool(name="cls_pool", bufs=3) as cls_pool,
            tc.tile_pool(name="exp_pool", bufs=3) as exp_pool,
            tc.tile_pool(name="n_pool", bufs=3) as n_pool,
            tc.tile_pool(name="scr_pool", bufs=3) as scr_pool,
            tc.tile_pool(name="const_pool", bufs=1) as const_pool,
            tc.tile_pool(name="stage_pool", bufs=1) as stage_pool,
            tc.tile_pool(name="psum_pool", bufs=1, space="PSUM") as psum_pool,
        ):
            ones_c = const_pool.tile([PC, 1], bf16)
            ones_d = const_pool.tile([PD, 1], bf16)
            nc.vector.memset(ones_c, 1.0)
            nc.vector.memset(ones_d, 1.0)

            stage_e = stage_pool.tile([1, r_pad], f32)
            stage_d = stage_pool.tile([1, r_pad], f32)

            r0 = 0
            for b, r_blk in enumerate(blocks):
                r1 = r0 + r_blk
                ctile = cls_pool.tile([PC, r_blk, JC], f8, tag="cls")
                nc.sync.dma_start(out=ctile, in_=cls_t.ap()[:, r0:r1, :])
                ntile = n_pool.tile([PD, r_blk, JD], bf16, tag="nn")
                nc.sync.dma_start(out=ntile, in_=n_t.ap()[:, r0:r1, :])
                ftile = n_pool.tile([PD, r_blk, JD], bf16, tag="ff")
                nc.sync.dma_start(out=ftile, in_=f_t.ap()[:, r0:r1, :])

                # expsum path: exp (ACT) -> 8->4 add tree (DVE, 2x bf16 mode)
                # -> 4 ones-matmul partition folds (PE) into psum_e
                etile = exp_pool.tile([PC, r_blk, JC], bf16, tag="exp")
                nc.scalar.activation(
                    out=etile, in_=ctile,
                    func=mybir.ActivationFunctionType.Exp,
                )
                t1 = scr_pool.tile([PC, r_blk, 4], bf16, tag="t1")
                nc.vector.tensor_add(t1, etile[:, :, 0:4], etile[:, :, 4:8])
                psum_e = psum_pool.tile([1, r_blk], f32, tag="pe")
                for j in range(4):
                    nc.tensor.matmul(
                        psum_e, ones_c, t1[:, :, j],
                        start=(j == 0), stop=(j == 3),
                    )

                # d2 path: tdif = n - 2f (DVE), h = tdif * n (GPSIMD, keeps
                # DVE under the ACT roofline), PE folds sum_d h into
                # psum_d = sum(n^2) - 2*f.n
                tdif = scr_pool.tile([PD, r_blk, JD], bf16, tag="td")
                nc.vector.tensor_sub(tdif, ntile, ftile)
                h = scr_pool.tile([PD, r_blk, JD], bf16, tag="h")
                nc.gpsimd.tensor_mul(h, tdif, ntile)
                psum_d = psum_pool.tile([1, r_blk], f32, tag="pd")
                for j in range(JD):
                    nc.tensor.matmul(
                        psum_d, ones_d, h[:, :, j],
                        start=(j == 0), stop=(j == JD - 1),
                    )

                # evacuate PSUM (DMA cannot read it) — partition-1 copies
                nc.vector.tensor_copy(stage_e[:, r0:r1], psum_e)
                nc.vector.tensor_copy(stage_d[:, r0:r1], psum_d)
                r0 = r1

            nc.sync.dma_start(out=out_t.ap()[0:1, :], in_=stage_e)
            nc.sync.dma_start(out=out_t.ap()[1:2, :], in_=stage_d)

    nc.compile()
    return nc


def _get_nc(key=None):
    if key is None:
        key = _CACHE.get("last_key")
        if key is None:
            key = _plan(-(-B * KMAX // N_CORES))
    if ("nc", key) not in _CACHE:
        _CACHE[("nc", key)] = _build_nc(key)
    _CACHE["last_key"] = key
    return _CACHE[("nc", key)]


def _run_device(nc, in_maps):
    global LAST_RESULTS
    from concourse import bass_utils

    trace = bool(int(os.environ.get("DOS_TRACE", "0")))
    last_exc = None
    for _attempt in range(3):
        try:
            results = bass_utils.run_bass_kernel_spmd(
                nc, in_maps, core_ids=list(range(N_CORES)), trace=trace
            )
            break
        except Exception as e:
            last_exc = e
            time.sleep(5)
    else:
        raise last_exc
    LAST_RESULTS = results
    return [r["out"] for r in results.results]


def kernel(deep_feats, n, w, cls_score, target, lengths):
    import ml_dtypes

    deep_feats = np.asarray(deep_feats, dtype=np.float32)
    n = np.asarray(n, dtype=np.float32)
    w = np.asarray(w, dtype=np.float32)
    cls_score = np.asarray(cls_score, dtype=np.float32)
    target = np.asarray(target).astype(np.int64)
    lengths = np.asarray(lengths).astype(np.int64)

    # packed stream of valid rows, ordered by (b, k)
    idx_b = np.repeat(np.arange(B), lengths)
    idx_k = np.concatenate([np.arange(l) for l in lengths])
    V = idx_b.shape[0]

    sizes = np.full(N_CORES, V // N_CORES, dtype=np.int64)
    sizes[: V % N_CORES] += 1
    starts = np.concatenate([[0], np.cumsum(sizes)])
    key = _plan(int(sizes.max()))
    r_pad = sum(key)

    f2 = 2.0 * deep_feats  # [B, D]
    in_maps = []
    for c in range(N_CORES):
        lo, hi = int(starts[c]), int(starts[c + 1])
        rb, rk = idx_b[lo:hi], idx_k[lo:hi]
        rc = hi - lo

        cls_rows = np.zeros((r_pad, C), dtype=np.float32)
        cls_rows[:rc] = cls_score[rb, rk]
        cls8 = np.ascontiguousarray(
            cls_rows.reshape(r_pad, PC, JC).transpose(1, 0, 2)
        ).astype(ml_dtypes.float8_e4m3fn)

        n_rows = np.zeros((r_pad, D), dtype=np.float32)
        n_rows[:rc] = n[rb, rk]
        nn2t = np.ascontiguousarray(
            n_rows.reshape(r_pad, PD, JD).transpose(1, 0, 2)
        ).astype(ml_dtypes.bfloat16)

        f_rows = np.zeros((r_pad, D), dtype=np.float32)
        f_rows[:rc] = f2[rb]
        f2r = np.ascontiguousarray(
            f_rows.reshape(r_pad, PD, JD).transpose(1, 0, 2)
        ).astype(ml_dtypes.bfloat16)

        in_maps.append({"cls8": cls8, "nn2t": nn2t, "f2r": f2r})

    outs = _run_device(_get_nc(key), in_maps)

    expsum = np.empty(V, dtype=np.float64)
    dpart = np.empty(V, dtype=np.float64)
    for c in range(N_CORES):
        lo, hi = int(starts[c]), int(starts[c + 1])
        o = np.asarray(outs[c], dtype=np.float64)  # [2, r_pad]
        expsum[lo:hi] = o[0, : hi - lo]
        dpart[lo:hi] = o[1, : hi - lo]

    # host tail in float64 over the packed stream
    ff2 = np.sum(deep_feats.astype(np.float64) ** 2, axis=1)  # [B]
    d2 = dpart + ff2[idx_b]
    dist = np.sqrt(np.maximum(d2, 0.0))
    wv = w[idx_b, idx_k].astype(np.float64)
    s = -wv * dist
    f_loss = float(np.sum(s))

    lse = np.log(np.maximum(expsum, 1e-300))
    cls_at = cls_score[idx_b, idx_k, target[idx_b]].astype(np.float64)
    ce = lse - cls_at

    # per-sample softmax of s over the ragged segments
    g_loss = 0.0
    for b in range(B):
        lo, hi = int(np.sum(lengths[:b])), int(np.sum(lengths[: b + 1]))
        sb = s[lo:hi]
        eb = np.exp(sb - sb.max())
        rho = eb / eb.sum()
        g_loss += float(np.sum(rho * ce[lo:hi]))

    return np.float32(f_loss + g_loss)


# revision 19
# speedup vs baseline: 2.8933x; 1.2139x over previous
"""DOSLoss Trainium2 kernel — ragged-packed, class-on-partition layout.

Full inputs in, scalar loss out. The two heavy per-row contractions run on
device; everything O(B*K) runs on host in float64.

Key ideas vs the naive per-sample kernel:
  * Ragged packing: only the sum(lengths) valid (b,k) rows are uploaded and
    processed (~half of B*Kmax for uniform lengths), load-balanced so every
    core gets ceil(V/8) rows regardless of per-sample lengths.
  * Class-on-partition layout: cls row r is stored as [125 partitions, 8]
    (c = p*8 + j, 1000 = 125*8 exactly). The ACT engine then does pure
    elementwise exp (8 elems/row of engine time, its roofline) with no
    per-instruction accumulator reads. A 2-level DVE pairwise add tree
    (bf16, 2x SIMD mode) folds 8 -> 2, and a ones-weight matmul on the
    otherwise idle PE contracts the 125 partitions into PSUM.
  * d2 via h = (n - 2f)*n summed by PE: host uploads n rows ([128, R, 2],
    d = p*2 + j) and 2*deep_feats replicated per row; DVE computes h in two
    tensor ops; PE ones-matmuls accumulate sum(n^2) - 2*f.n into PSUM in the
    same accumulation group. Host adds ||f||^2 (exact, fp64).
  * dtypes: cls in fp8 e4m3 (exp(x) feeds a 1000-term sum; rounding is
    ~0.04 absolute on the logit -> ~2e-3 on lse, irrelevant at the 2e-2
    gate), n/2f in bf16. Halves/quarters DMA bytes; ACT cost is dtype-blind.
  * Per-block PE results land in per-partition rows of two persistent PSUM
    tiles ([NBLK, R_blk], block b -> partition b); two partition-parallel
    DVE copies at the tail evacuate them to SBUF for one small output DMA
    (DMA cannot read PSUM directly).

Per-block (424 rows) engine budget from the TimelineSim cost model:
ACT 2.92us (bottleneck), DMA 2.2us, DVE ~2.2us, PE ~1.4us.
"""

import os
import time

import numpy as np

B, KMAX, D, C = 64, 512, 256, 1000
N_CORES = 8
PC = 125  # class partitions: C = PC * 8
JC = 8
PD = 128  # d partitions: D = PD * 2
JD = 2
MAX_RBLK = 448  # matmul moving-dim cap is 512; keep a multiple of 8 below it
TAIL_ROWS = 128  # rows handled by the rows-on-partition accum-out tail path

_CACHE = {}
LAST_RESULTS = None  # BassKernelResults of the most recent device run


def _plan(v_max):
    """Block plan for v_max rows per core: tuple of block row counts.

    Small first block shortens pipeline fill (first exp starts after a small
    cls DMA); small last block shortens the post-ACT tail chain (fold +
    matmul + PSUM copies of the final block). Middle blocks bounded by the
    512-row matmul moving-dim cap.
    """
    if v_max <= 640:
        return ((-(-v_max // 8) * 8,), 0)
    # last TAIL_ROWS rows run in the rows-on-partition accum layout; the
    # last PSUM-path block is small so its fold->copy->DMA chain doesn't
    # stretch the kernel tail
    rem = v_max - TAIL_ROWS
    ramp = (128, 256)
    down = (96,)
    rem -= sum(ramp) + sum(down)
    nmid = max(1, -(-rem // MAX_RBLK))
    mid = -(-rem // (nmid * 8)) * 8
    return (ramp + (mid,) * nmid + down, TAIL_ROWS)


def _build_nc(blocks, tail_rows):
    import concourse.bacc as bacc
    import concourse.mybir as mybir
    import concourse.tile as tile

    f32 = mybir.dt.float32
    bf16 = mybir.dt.bfloat16
    f8 = mybir.dt.float8e4
    r_main = sum(blocks)

    nc = bacc.Bacc("TRN2", target_bir_lowering=False, debug=False)

    cls_t = nc.dram_tensor("cls8", [PC, r_main, JC], f8, kind="ExternalInput")
    nf_t = nc.dram_tensor("nf", [PD, r_main, 2 * JD], bf16, kind="ExternalInput")
    out_t = nc.dram_tensor("out", [2, r_main], f32, kind="ExternalOutput")
    if tail_rows:
        ctl_t = nc.dram_tensor(
            "cls_tl", [tail_rows, C], f8, kind="ExternalInput"
        )
        nftl_t = nc.dram_tensor(
            "nf_tl", [tail_rows, 2 * D], bf16, kind="ExternalInput"
        )
        otl_t = nc.dram_tensor(
            "out_tl", [tail_rows, 2], f32, kind="ExternalOutput"
        )

    with tile.TileContext(nc) as tc:
        with (
            tc.tile_pool(name="cls_pool", bufs=3) as cls_pool,
            tc.tile_pool(name="exp_pool", bufs=3) as exp_pool,
            tc.tile_pool(name="n_pool", bufs=3) as n_pool,
            tc.tile_pool(name="scr_pool", bufs=3) as scr_pool,
            tc.tile_pool(name="const_pool", bufs=1) as const_pool,
            tc.tile_pool(name="stage_pool", bufs=1) as stage_pool,
            tc.tile_pool(name="psum_pool", bufs=3, space="PSUM") as psum_pool,
        ):
            ones_c = const_pool.tile([PC, 1], bf16)
            ones_d = const_pool.tile([PD, 1], bf16)
            nc.vector.memset(ones_c, 1.0)
            nc.vector.memset(ones_d, 1.0)

            stage_e = stage_pool.tile([1, r_main], f32)
            stage_d = stage_pool.tile([1, r_main], f32)

            nblk = len(blocks)
            deferred = []  # (r0, r1, psum_e, psum_d) awaiting evacuation
            prefix_end = sum(blocks[:-2]) if nblk > 2 else 0
            # cls DMAs for the first two blocks issue back-to-back before
            # anything else: the HWDGE pipeline (625ns/DMA) otherwise delays
            # block 1's exp at the pipeline ramp
            pre_ctiles = {}
            pr0 = 0
            for b in range(min(2, nblk)):
                pr1 = pr0 + blocks[b]
                ct = cls_pool.tile([PC, blocks[b], JC], f8, tag=f"cls{b}")
                nc.sync.dma_start(out=ct, in_=cls_t.ap()[:, pr0:pr1, :])
                pre_ctiles[b] = ct
                pr0 = pr1
            ctl = nftl = None
            r0 = 0
            for b, r_blk in enumerate(blocks):
                r1 = r0 + r_blk
                if b in pre_ctiles:
                    ctile = pre_ctiles[b]
                else:
                    ctile = cls_pool.tile([PC, r_blk, JC], f8, tag="cls")
                    nc.sync.dma_start(out=ctile, in_=cls_t.ap()[:, r0:r1, :])
                if tail_rows and b == min(2, nblk - 1):
                    # tail inputs load mid-ramp: issuing them any earlier
                    # wedges 2x625ns of HWDGE work between the ramp cls
                    # fetches and stalls the exp stream
                    ctl = cls_pool.tile([tail_rows, C], f8)
                    nc.sync.dma_start(out=ctl, in_=ctl_t.ap())
                    nftl = n_pool.tile([tail_rows, 2 * D], bf16)
                    nc.sync.dma_start(out=nftl, in_=nftl_t.ap())
                # n and 2f ride in one interleaved tensor -> one DMA issue
                # (the SP sequencer costs 565ns per issue and otherwise
                # delays the cls stream)
                nftile = n_pool.tile([PD, r_blk, 2 * JD], bf16, tag="nf")
                nc.sync.dma_start(out=nftile, in_=nf_t.ap()[:, r0:r1, :])
                ntile = nftile[:, :, 0:JD]
                ftile = nftile[:, :, JD : 2 * JD]

                # expsum path: exp (ACT), then the otherwise-idle PE folds
                # all 8 class columns and the 125 partitions with 8
                # accumulating ones-matmuls into psum_e
                etile = exp_pool.tile([PC, r_blk, JC], bf16, tag="exp")
                nc.scalar.activation(
                    out=etile, in_=ctile,
                    func=mybir.ActivationFunctionType.Exp,
                )
                psum_e = psum_pool.tile([1, r_blk], f32, tag="pe")
                for j in range(JC):
                    nc.tensor.matmul(
                        psum_e, ones_c, etile[:, :, j],
                        start=(j == 0), stop=(j == JC - 1),
                    )

                # d2 path: tdif = n - 2f, h = tdif * n (DVE, 2x bf16 mode);
                # PE folds sum_d h into psum_d = sum(n^2) - 2*f.n
                tdif = scr_pool.tile([PD, r_blk, JD], bf16, tag="td")
                nc.vector.tensor_sub(tdif, ntile, ftile)
                h = scr_pool.tile([PD, r_blk, JD], bf16, tag="h")
                nc.vector.tensor_mul(h, tdif, ntile)
                psum_d = psum_pool.tile([1, r_blk], f32, tag="pd")
                for j in range(JD):
                    nc.tensor.matmul(
                        psum_d, ones_d, h[:, :, j],
                        start=(j == 0), stop=(j == JD - 1),
                    )

                # evacuate PSUM (DMA cannot read it; engine streams are
                # in-order). Copies lag one block behind compute on DVE so
                # they never sit between exp(b)'s fold consumers.
                deferred.append((r0, r1, psum_e, psum_d))
                if len(deferred) > 1:
                    c0, c1, pe_t, pd_t = deferred.pop(0)
                    nc.vector.tensor_copy(stage_e[:, c0:c1], pe_t)
                    nc.vector.tensor_copy(stage_d[:, c0:c1], pd_t)
                    if c1 == prefix_end:
                        # everything before the two small closing blocks
                        # ships while they are still in flight
                        nc.sync.dma_start(
                            out=out_t.ap()[0:1, :c1], in_=stage_e[:, :c1]
                        )
                        nc.sync.dma_start(
                            out=out_t.ap()[1:2, :c1], in_=stage_d[:, :c1]
                        )
                r0 = r1

            if tail_rows:
                # tail path: rows on partitions; exp's accumulator gives the
                # per-row class sum directly (no PSUM round-trip), the d2
                # column is a short DVE chain. One tiny DMA, no fold/copy on
                # the critical tail.
                stage_tl = stage_pool.tile([tail_rows, 2], f32)
                etl = exp_pool.tile([tail_rows, C], bf16)
                nc.scalar.activation(
                    out=etl, in_=ctl,
                    func=mybir.ActivationFunctionType.Exp,
                    accum_out=stage_tl[:, 0:1],
                )
                ttd = scr_pool.tile([tail_rows, D], bf16, tag="ttd")
                nc.vector.tensor_sub(ttd, nftl[:, 0:D], nftl[:, D : 2 * D])
                tth = scr_pool.tile([tail_rows, D], bf16, tag="tth")
                nc.vector.tensor_mul(tth, ttd, nftl[:, 0:D])
                with nc.allow_low_precision("f32 accumulate"):
                    nc.vector.reduce_sum(
                        out=stage_tl[:, 1:2], in_=tth,
                        axis=mybir.AxisListType.X,
                    )
                nc.scalar.dma_start(out=otl_t.ap(), in_=stage_tl)

            for r0, r1, pe_t, pd_t in deferred:
                nc.vector.tensor_copy(stage_e[:, r0:r1], pe_t)
                nc.vector.tensor_copy(stage_d[:, r0:r1], pd_t)
            sfx = prefix_end
            nc.sync.dma_start(out=out_t.ap()[0:1, sfx:], in_=stage_e[:, sfx:])
            nc.sync.dma_start(out=out_t.ap()[1:2, sfx:], in_=stage_d[:, sfx:])

    nc.compile()
    return nc


def _get_nc(key=None):
    if key is None:
        key = _CACHE.get("last_key")
        if key is None:
            key = _plan(-(-B * KMAX // N_CORES))
    if ("nc", key) not in _CACHE:
        _CACHE[("nc", key)] = _build_nc(*key)
    _CACHE["last_key"] = key
    return _CACHE[("nc", key)]


def _run_device(nc, in_maps):
    global LAST_RESULTS
    from concourse import bass_utils

    trace = bool(int(os.environ.get("DOS_TRACE", "0")))
    last_exc = None
    for _attempt in range(3):
        try:
            results = bass_utils.run_bass_kernel_spmd(
                nc, in_maps, core_ids=list(range(N_CORES)), trace=trace
            )
            break
        except Exception as e:
            last_exc = e
            time.sleep(5)
    else:
        raise last_exc
    LAST_RESULTS = results
    return list(results.results)


def kernel(deep_feats, n, w, cls_score, target, lengths):
    import ml_dtypes

    deep_feats = np.asarray(deep_feats, dtype=np.float32)
    n = np.asarray(n, dtype=np.float32)
    w = np.asarray(w, dtype=np.float32)
    cls_score = np.asarray(cls_score, dtype=np.float32)
    target = np.asarray(target).astype(np.int64)
    lengths = np.asarray(lengths).astype(np.int64)

    # packed stream of valid rows, ordered by (b, k)
    idx_b = np.repeat(np.arange(B), lengths)
    idx_k = np.concatenate([np.arange(l) for l in lengths])
    V = idx_b.shape[0]

    sizes = np.full(N_CORES, V // N_CORES, dtype=np.int64)
    sizes[: V % N_CORES] += 1
    starts = np.concatenate([[0], np.cumsum(sizes)])
    key = _plan(int(sizes.max()))
    blocks, tail_rows = key
    r_main = sum(blocks)
    r_pad = r_main + tail_rows

    f2 = 2.0 * deep_feats  # [B, D]
    in_maps = []
    for c in range(N_CORES):
        lo, hi = int(starts[c]), int(starts[c + 1])
        rb, rk = idx_b[lo:hi], idx_k[lo:hi]
        rc = hi - lo

        cls_rows = np.zeros((r_pad, C), dtype=np.float32)
        cls_rows[:rc] = cls_score[rb, rk]
        n_rows = np.zeros((r_pad, D), dtype=np.float32)
        n_rows[:rc] = n[rb, rk]
        f_rows = np.zeros((r_pad, D), dtype=np.float32)
        f_rows[:rc] = f2[rb]

        cls8 = np.ascontiguousarray(
            cls_rows[:r_main].reshape(r_main, PC, JC).transpose(1, 0, 2)
        ).astype(ml_dtypes.float8_e4m3fn)
        nf = np.empty((PD, r_main, 2 * JD), dtype=np.float32)
        nf[:, :, 0:JD] = (
            n_rows[:r_main].reshape(r_main, PD, JD).transpose(1, 0, 2)
        )
        nf[:, :, JD:] = (
            f_rows[:r_main].reshape(r_main, PD, JD).transpose(1, 0, 2)
        )
        nf = np.ascontiguousarray(nf).astype(ml_dtypes.bfloat16)
        im = {"cls8": cls8, "nf": nf}
        if tail_rows:
            im["cls_tl"] = cls_rows[r_main:].astype(ml_dtypes.float8_e4m3fn)
            nftl = np.concatenate(
                [n_rows[r_main:], f_rows[r_main:]], axis=1
            )
            im["nf_tl"] = nftl.astype(ml_dtypes.bfloat16)
        in_maps.append(im)

    outs = _run_device(_get_nc(key), in_maps)

    expsum = np.empty(V, dtype=np.float64)
    dpart = np.empty(V, dtype=np.float64)
    for c in range(N_CORES):
        lo, hi = int(starts[c]), int(starts[c + 1])
        o = np.asarray(outs[c]["out"], dtype=np.float64)  # [2, r_main]
        full = np.empty((2, r_pad), dtype=np.float64)
        full[:, :r_main] = o
        if tail_rows:
            otl = np.asarray(outs[c]["out_tl"], dtype=np.float64)
            full[:, r_main:] = otl.T
        expsum[lo:hi] = full[0, : hi - lo]
        dpart[lo:hi] = full[1, : hi - lo]

    # host tail in float64 over the packed stream
    ff2 = np.sum(deep_feats.astype(np.float64) ** 2, axis=1)  # [B]
    d2 = dpart + ff2[idx_b]
    dist = np.sqrt(np.maximum(d2, 0.0))
    wv = w[idx_b, idx_k].astype(np.float64)
    s = -wv * dist
    f_loss = float(np.sum(s))

    lse = np.log(np.maximum(expsum, 1e-300))
    cls_at = cls_score[idx_b, idx_k, target[idx_b]].astype(np.float64)
    ce = lse - cls_at

    # per-sample softmax of s over the ragged segments
    g_loss = 0.0
    for b in range(B):
        lo, hi = int(np.sum(lengths[:b])), int(np.sum(lengths[: b + 1]))
        sb = s[lo:hi]
        eb = np.exp(sb - sb.max())
        rho = eb / eb.sum()
        g_loss += float(np.sum(rho * ce[lo:hi]))

    return np.float32(f_loss + g_loss)


# revision 25
# speedup vs baseline: 2.9391x; 1.0159x over previous
"""DOSLoss Trainium2 kernel — ragged-packed, class-on-partition layout.

Full inputs in, scalar loss out. The two heavy per-row contractions run on
device; everything O(B*K) runs on host in float64.

Key ideas vs the naive per-sample kernel:
  * Ragged packing: only the sum(lengths) valid (b,k) rows are uploaded and
    processed (~half of B*Kmax for uniform lengths), load-balanced so every
    core gets ceil(V/8) rows regardless of per-sample lengths.
  * Class-on-partition layout: cls row r is stored as [125 partitions, 8]
    (c = p*8 + j, 1000 = 125*8 exactly). The ACT engine then does pure
    elementwise exp (8 elems/row of engine time, its roofline) with no
    per-instruction accumulator reads. A 2-level DVE pairwise add tree
    (bf16, 2x SIMD mode) folds 8 -> 2, and a ones-weight matmul on the
    otherwise idle PE contracts the 125 partitions into PSUM.
  * d2 via h = (n - 2f)*n summed by PE: host uploads n rows ([128, R, 2],
    d = p*2 + j) and 2*deep_feats replicated per row; DVE computes h in two
    tensor ops; PE ones-matmuls accumulate sum(n^2) - 2*f.n into PSUM in the
    same accumulation group. Host adds ||f||^2 (exact, fp64).
  * dtypes: cls in fp8 e4m3 (exp(x) feeds a 1000-term sum; rounding is
    ~0.04 absolute on the logit -> ~2e-3 on lse, irrelevant at the 2e-2
    gate), n/2f in bf16. Halves/quarters DMA bytes; ACT cost is dtype-blind.
  * Per-block PE results land in per-partition rows of two persistent PSUM
    tiles ([NBLK, R_blk], block b -> partition b); two partition-parallel
    DVE copies at the tail evacuate them to SBUF for one small output DMA
    (DMA cannot read PSUM directly).

Per-block (424 rows) engine budget from the TimelineSim cost model:
ACT 2.92us (bottleneck), DMA 2.2us, DVE ~2.2us, PE ~1.4us.
"""

import os
import time

import numpy as np

B, KMAX, D, C = 64, 512, 256, 1000
N_CORES = 8
PC = 125  # class partitions: C = PC * 8
JC = 8
PD = 128  # d partitions: D = PD * 2
JD = 2
MAX_RBLK = 448  # matmul moving-dim cap is 512; keep a multiple of 8 below it
TAIL_ROWS = 128  # rows handled by the rows-on-partition accum-out tail path

_CACHE = {}
LAST_RESULTS = None  # BassKernelResults of the most recent device run


def _plan(v_max):
    """Block plan for v_max rows per core: tuple of block row counts.

    Small first block shortens pipeline fill (first exp starts after a small
    cls DMA); small last block shortens the post-ACT tail chain (fold +
    matmul + PSUM copies of the final block). Middle blocks bounded by the
    512-row matmul moving-dim cap.
    """
    if v_max <= 640:
        return (((-(-v_max // 8) * 8,),), 0)
    # Plan = exp chunks; each chunk is one cls DMA + one exp instruction
    # (185ns init amortized) split into <=MAX_RBLK fold blocks (matmul
    # moving-dim cap). Ramp-up chunks keep the exp stream fed during
    # pipeline fill; the small closing block shortens the fold->copy->DMA
    # tail; the last TAIL_ROWS rows use the rows-on-partition accum layout.
    rem = v_max - TAIL_ROWS
    ramp = ((128,), (256,))
    down = ((96,),)
    rem -= 384 + 96
    nmid = max(1, -(-rem // MAX_RBLK))
    mid = -(-rem // (nmid * 8)) * 8
    chunks = tuple((mid,) for _ in range(nmid))
    return (ramp + chunks + down, TAIL_ROWS)


def _build_nc(chunks, tail_rows):
    import concourse.bacc as bacc
    import concourse.mybir as mybir
    import concourse.tile as tile

    f32 = mybir.dt.float32
    bf16 = mybir.dt.bfloat16
    f8 = mybir.dt.float8e4
    blocks = [b for ch in chunks for b in ch]
    csizes = [sum(ch) for ch in chunks]
    r_main = sum(csizes)
    nblk = len(blocks)

    nc = bacc.Bacc("TRN2", target_bir_lowering=False, debug=False)

    cls_t = nc.dram_tensor("cls8", [PC, r_main, JC], f8, kind="ExternalInput")
    nf_t = nc.dram_tensor("nf", [PD, r_main, 2 * JD], bf16, kind="ExternalInput")
    out_t = nc.dram_tensor("out", [2, r_main], f32, kind="ExternalOutput")
    if tail_rows:
        ctl_t = nc.dram_tensor(
            "cls_tl", [tail_rows, C], f8, kind="ExternalInput"
        )
        nftl_t = nc.dram_tensor(
            "nf_tl", [tail_rows, 2 * D], bf16, kind="ExternalInput"
        )
        otl_t = nc.dram_tensor(
            "out_tl", [tail_rows, 2], f32, kind="ExternalOutput"
        )

    with tile.TileContext(nc) as tc:
        with (
            tc.tile_pool(name="cls_pool", bufs=4) as cls_pool,
            tc.tile_pool(name="exp_pool", bufs=3) as exp_pool,
            tc.tile_pool(name="n_pool", bufs=3) as n_pool,
            tc.tile_pool(name="scr_pool", bufs=3) as scr_pool,
            tc.tile_pool(name="const_pool", bufs=1) as const_pool,
            tc.tile_pool(name="stage_pool", bufs=1) as stage_pool,
            tc.tile_pool(name="psum_pool", bufs=3, space="PSUM") as psum_pool,
        ):
            ones_c = const_pool.tile([PC, 1], bf16)
            ones_d = const_pool.tile([PD, 1], bf16)
            nc.vector.memset(ones_c, 1.0)
            nc.vector.memset(ones_d, 1.0)

            stage = stage_pool.tile([1, 2 * r_main], f32)

            # cls DMAs run two chunks ahead of the nf stream so exp(b) never
            # waits on a fetch that queued behind nf(b-1) on the DMA engines
            cstarts = [sum(csizes[:i]) for i in range(len(csizes))]
            ctiles = {}

            def issue_cls(ci):
                c0 = cstarts[ci]
                c1 = c0 + csizes[ci]
                ct = cls_pool.tile([PC, csizes[ci], JC], f8, tag=f"cls{ci % 4}")
                nc.sync.dma_start(out=ct, in_=cls_t.ap()[:, c0:c1, :])
                ctiles[ci] = ct

            for ci in range(min(3, len(chunks))):
                issue_cls(ci)

            deferred = []  # (r0, r1, psum_e, psum_d) awaiting evacuation
            prefix_end = sum(blocks[:-2]) if nblk > 2 else 0
            ctl = nftl = None
            b_idx = 0
            r0 = 0
            for ci, ch in enumerate(chunks):
                if ci + 3 < len(chunks):
                    issue_cls(ci + 3)
                if tail_rows and ci == min(2, len(chunks) - 1):
                    # tail inputs load mid-ramp: any earlier and their HWDGE
                    # slots delay the ramp cls fetches
                    ctl = cls_pool.tile([tail_rows, C], f8)
                    nc.sync.dma_start(out=ctl, in_=ctl_t.ap())
                    nftl = n_pool.tile([tail_rows, 2 * D], bf16)
                    nc.sync.dma_start(out=nftl, in_=nftl_t.ap())

                csz = sum(ch)
                c1 = r0 + csz
                ctile = ctiles.pop(ci)
                nftile = n_pool.tile([PD, csz, 2 * JD], bf16, tag="nf")
                nc.sync.dma_start(out=nftile, in_=nf_t.ap()[:, r0:c1, :])

                # one exp instruction per chunk
                etile = exp_pool.tile([PC, csz, JC], bf16, tag="exp")
                nc.scalar.activation(
                    out=etile, in_=ctile,
                    func=mybir.ActivationFunctionType.Exp,
                )
                # d-path per chunk on DVE (2x bf16 mode)
                ntile = nftile[:, :, 0:JD]
                ftile = nftile[:, :, JD : 2 * JD]
                tdif = scr_pool.tile([PD, csz, JD], bf16, tag="td")
                nc.vector.tensor_sub(tdif, ntile, ftile)
                h = scr_pool.tile([PD, csz, JD], bf16, tag="h")
                nc.vector.tensor_mul(h, tdif, ntile)

                # fold blocks: PE contracts partitions into PSUM
                s0 = 0
                for r_blk in ch:
                    s1 = s0 + r_blk
                    psum_e = psum_pool.tile([1, r_blk], f32, tag="pe")
                    for j in range(JC):
                        nc.tensor.matmul(
                            psum_e, ones_c, etile[:, s0:s1, j],
                            start=(j == 0), stop=(j == JC - 1),
                        )
                    psum_d = psum_pool.tile([1, r_blk], f32, tag="pd")
                    for j in range(JD):
                        nc.tensor.matmul(
                            psum_d, ones_d, h[:, s0:s1, j],
                            start=(j == 0), stop=(j == JD - 1),
                        )
                    # evacuate PSUM (DMA cannot read it; engine streams are
                    # in-order). Copies lag one block behind on DVE.
                    deferred.append((r0 + s0, r0 + s1, psum_e, psum_d))
                    if len(deferred) > 1:
                        d0, d1, pe_t, pd_t = deferred.pop(0)
                        nc.vector.tensor_copy(stage[:, d0:d1], pe_t)
                        nc.vector.tensor_copy(
                            stage[:, r_main + d0 : r_main + d1], pd_t
                        )
                        if d1 == prefix_end:
                            # one 2-descriptor DMA ships both quantities'
                            # prefixes while the closing blocks are in flight
                            pre = stage[:, :].rearrange(
                                "p (q r) -> p q r", r=r_main
                            )[:, :, :d1]
                            nc.sync.dma_start(
                                out=out_t.ap()[:, :d1], in_=pre
                            )
                    s0 = s1
                    b_idx += 1
                r0 = c1

            if tail_rows:
                # tail path: rows on partitions; exp's accumulator gives the
                # per-row class sum directly (no PSUM round-trip), the d2
                # column is a short DVE chain, one tiny SP DMA ends the
                # kernel with no fold/copy on the critical tail.
                stage_tl = stage_pool.tile([tail_rows, 2], f32)
                etl = exp_pool.tile([tail_rows, C], bf16)
                nc.scalar.activation(
                    out=etl, in_=ctl,
                    func=mybir.ActivationFunctionType.Exp,
                    accum_out=stage_tl[:, 0:1],
                )
                ttd = scr_pool.tile([tail_rows, D], bf16, tag="ttd")
                nc.vector.tensor_sub(ttd, nftl[:, 0:D], nftl[:, D : 2 * D])
                tth = scr_pool.tile([tail_rows, D], bf16, tag="tth")
                nc.vector.tensor_mul(tth, ttd, nftl[:, 0:D])
                with nc.allow_low_precision("f32 accumulate"):
                    nc.vector.reduce_sum(
                        out=stage_tl[:, 1:2], in_=tth,
                        axis=mybir.AxisListType.X,
                    )
                nc.sync.dma_start(out=otl_t.ap(), in_=stage_tl)

            for d0, d1, pe_t, pd_t in deferred:
                nc.scalar.copy(stage[:, d0:d1], pe_t)
                nc.scalar.copy(stage[:, r_main + d0 : r_main + d1], pd_t)
            sfx = prefix_end
            suf = stage[:, :].rearrange("p (q r) -> p q r", r=r_main)[
                :, :, sfx:
            ]
            nc.sync.dma_start(out=out_t.ap()[:, sfx:], in_=suf)

    nc.compile()
    return nc


def _get_nc(key=None):
    if key is None:
        key = _CACHE.get("last_key")
        if key is None:
            key = _plan(-(-B * KMAX // N_CORES))
    if ("nc", key) not in _CACHE:
        _CACHE[("nc", key)] = _build_nc(*key)
    _CACHE["last_key"] = key
    return _CACHE[("nc", key)]


def _run_device(nc, in_maps):
    global LAST_RESULTS
    from concourse import bass_utils

    trace = bool(int(os.environ.get("DOS_TRACE", "0")))
    last_exc = None
    for _attempt in range(3):
        try:
            results = bass_utils.run_bass_kernel_spmd(
                nc, in_maps, core_ids=list(range(N_CORES)), trace=trace
            )
            break
        except Exception as e:
            last_exc = e
            time.sleep(5)
    else:
        raise last_exc
    LAST_RESULTS = results
    return list(results.results)


def kernel(deep_feats, n, w, cls_score, target, lengths):
    import ml_dtypes

    deep_feats = np.asarray(deep_feats, dtype=np.float32)
    n = np.asarray(n, dtype=np.float32)
    w = np.asarray(w, dtype=np.float32)
    cls_score = np.asarray(cls_score, dtype=np.float32)
    target = np.asarray(target).astype(np.int64)
    lengths = np.asarray(lengths).astype(np.int64)

    # packed stream of valid rows, ordered by (b, k)
    idx_b = np.repeat(np.arange(B), lengths)
    idx_k = np.concatenate([np.arange(l) for l in lengths])
    V = idx_b.shape[0]

    sizes = np.full(N_CORES, V // N_CORES, dtype=np.int64)
    sizes[: V % N_CORES] += 1
    starts = np.concatenate([[0], np.cumsum(sizes)])
    key = _plan(int(sizes.max()))
    chunks, tail_rows = key
    r_main = sum(sum(ch) for ch in chunks)
    r_pad = r_main + tail_rows

    f2 = 2.0 * deep_feats  # [B, D]
    in_maps = []
    for c in range(N_CORES):
        lo, hi = int(starts[c]), int(starts[c + 1])
        rb, rk = idx_b[lo:hi], idx_k[lo:hi]
        rc = hi - lo

        cls_rows = np.zeros((r_pad, C), dtype=np.float32)
        cls_rows[:rc] = cls_score[rb, rk]
        n_rows = np.zeros((r_pad, D), dtype=np.float32)
        n_rows[:rc] = n[rb, rk]
        f_rows = np.zeros((r_pad, D), dtype=np.float32)
        f_rows[:rc] = f2[rb]

        cls8 = np.ascontiguousarray(
            cls_rows[:r_main].reshape(r_main, PC, JC).transpose(1, 0, 2)
        ).astype(ml_dtypes.float8_e4m3fn)
        nf = np.empty((PD, r_main, 2 * JD), dtype=np.float32)
        nf[:, :, 0:JD] = (
            n_rows[:r_main].reshape(r_main, PD, JD).transpose(1, 0, 2)
        )
        nf[:, :, JD:] = (
            f_rows[:r_main].reshape(r_main, PD, JD).transpose(1, 0, 2)
        )
        nf = np.ascontiguousarray(nf).astype(ml_dtypes.bfloat16)
        im = {"cls8": cls8, "nf": nf}
        if tail_rows:
            im["cls_tl"] = cls_rows[r_main:].astype(ml_dtypes.float8_e4m3fn)
            nftl = np.concatenate(
                [n_rows[r_main:], f_rows[r_main:]], axis=1
            )
            im["nf_tl"] = nftl.astype(ml_dtypes.bfloat16)
        in_maps.append(im)

    outs = _run_device(_get_nc(key), in_maps)

    expsum = np.empty(V, dtype=np.float64)
    dpart = np.empty(V, dtype=np.float64)
    for c in range(N_CORES):
        lo, hi = int(starts[c]), int(starts[c + 1])
        o = np.asarray(outs[c]["out"], dtype=np.float64)  # [2, r_main]
        full = np.empty((2, r_pad), dtype=np.float64)
        full[:, :r_main] = o
        if tail_rows:
            otl = np.asarray(outs[c]["out_tl"], dtype=np.float64)
            full[:, r_main:] = otl.T
        expsum[lo:hi] = full[0, : hi - lo]
        dpart[lo:hi] = full[1, : hi - lo]

    # host tail in float64 over the packed stream
    ff2 = np.sum(deep_feats.astype(np.float64) ** 2, axis=1)  # [B]
    d2 = dpart + ff2[idx_b]
    dist = np.sqrt(np.maximum(d2, 0.0))
    wv = w[idx_b, idx_k].astype(np.float64)
    s = -wv * dist
    f_loss = float(np.sum(s))

    lse = np.log(np.maximum(expsum, 1e-300))
    cls_at = cls_score[idx_b, idx_k, target[idx_b]].astype(np.float64)
    ce = lse - cls_at

    # per-sample softmax of s over the ragged segments
    g_loss = 0.0
    for b in range(B):
        lo, hi = int(np.sum(lengths[:b])), int(np.sum(lengths[: b + 1]))
        sb = s[lo:hi]
        eb = np.exp(sb - sb.max())
        rho = eb / eb.sum()
        g_loss += float(np.sum(rho * ce[lo:hi]))

    return np.float32(f_loss + g_loss)
